# revision 9
# baseline (speedup 1.0000x reference)
"""MeanAggregatorSparse on 8 Trainium2 NeuronCores.

out = concat(self_feat, segment_mean(nbr_feat, idx)) @ W

Strategy: shard NODES across the 8 cores (6272 nodes = 49 windows of 128
each). Edges are bucketed host-side to the core/window owning their target
node (this is the sharding step - each core receives exactly the edges it
needs, so no collective is required). On device, each 128-edge tile builds a
weighted one-hot matrix oh[e, n] = (idx_local[e] == n) * (1/count[idx[e]])
with a single DVE tensor_scalar op, and the PE contracts
  S_T[feat, nodes] += feat_tile[edges, feat].T @ oh[edges, nodes]
accumulating a full 128-node window in PSUM. The weighted one-hot folds the
mean division into the matmul. A second pair of matmuls per window computes
  out[nodes, :] = agg[nodes, :] @ W_bot + self[nodes, :] @ W_top
directly from the transposed accumulator (no transposes needed anywhere).
"""

import numpy as np

P = 128
N_NODES = 50000
D_FEAT = 128
OUT_DIM = 128
N_CORES = 8
WPC = 49                        # node windows per core
NPC = WPC * P                   # nodes per core (6272)
NODES_PAD = N_CORES * NPC       # 50176
N_WIN = N_CORES * WPC           # 392


def _patch_tile_drain():
    """This walrus build caps sync waits at 1 per TPB_CTRL instruction, but
    TileContext's tail piles every outstanding sem wait onto a single Drain.
    Split them into one Drain per wait."""
    import bass_rust
    import concourse.tile as tile
    from concourse.tile import ScopedClock

    if getattr(tile.TileContext, "_drain_patched", False):
        return

    def _drain_and_barrier(self, tick_clock, wait_clock):
        nc = self.nc
        drain_inst = nc.sync.drain()
        wait_clock.add_sem_waits(
            drain_inst.ins, ScopedClock({None: tick_clock.global_clock})
        )
        si = drain_inst.ins.sync_info
        if si is not None and len(si.on_wait) > 1:
            waits = list(si.on_wait)
            si.on_wait = waits[:1]
            drain_inst.ins.sync_info = si
            for w in waits[1:]:
                d2 = nc.sync.drain()
                d2.ins.sync_info = bass_rust.SyncInfo(on_wait=[w], on_update=[])

        nc.all_engine_barrier()
        assert self.sems is not None
        popped = nc._tile_sem_poison_stack.pop()
        assert popped is self._sem_poison
        nc.clear_and_free_semaphores(list(self.sems.allocated().values()))
        nc.all_engine_barrier()

    tile.TileContext._drain_and_barrier = _drain_and_barrier
    tile.TileContext._drain_patched = True


_prog_cache = {}


def _build_program(T):
    """Build the SPMD Bass program for per-window edge capacity T."""
    import concourse.mybir as mybir
    import concourse.tile as tile
    from concourse import bacc
    from contextlib import ExitStack

    f32 = mybir.dt.float32
    NT = T // P

    nc = bacc.Bacc(
        "TRN2", target_bir_lowering=False, debug=False, num_devices=N_CORES
    )
    feats = nc.declare_dram_parameter("feats", [WPC * T, D_FEAT], f32, isOutput=False)
    meta = nc.declare_dram_parameter("meta", [P, WPC * NT * 2], f32, isOutput=False)
    selfT = nc.declare_dram_parameter("selfT", [P, NPC], f32, isOutput=False)
    wmat = nc.declare_dram_parameter("wmat", [2 * D_FEAT, OUT_DIM], f32, isOutput=False)
    iota = nc.declare_dram_parameter("iota", [P, P], f32, isOutput=False)
    outp = nc.declare_dram_parameter("outp", [NPC, OUT_DIM], f32, isOutput=True)

    with tile.TileContext(nc) as tc, ExitStack() as ctx:
        const = ctx.enter_context(tc.tile_pool(name="const", bufs=1))
        selft = const.tile([P, NPC], f32)
        nc.sync.dma_start(selft[:], selfT[:])
        wtop = const.tile([P, OUT_DIM], f32, tag="wtop")
        nc.sync.dma_start(wtop[:], wmat[0:P, :])
        wbot = const.tile([P, OUT_DIM], f32, tag="wbot")
        nc.sync.dma_start(wbot[:], wmat[P : 2 * P, :])
        metat = const.tile([P, WPC * NT * 2], f32)
        nc.sync.dma_start(metat[:], meta[:])
        iotat = const.tile([P, P], f32)
        nc.sync.dma_start(iotat[:], iota[:])

        featp = ctx.enter_context(tc.tile_pool(name="featp", bufs=3))
        ohp = ctx.enter_context(tc.tile_pool(name="ohp", bufs=4))
        aggp = ctx.enter_context(tc.tile_pool(name="aggp", bufs=2))
        obp = ctx.enter_context(tc.tile_pool(name="obp", bufs=2))
        psS_p = ctx.enter_context(tc.tile_pool(name="psS", bufs=2, space="PSUM"))
        psO_p = ctx.enter_context(tc.tile_pool(name="psO", bufs=2, space="PSUM"))

        # feats rows are laid out partition-major per window: row p*NT+k of
        # window j holds edge (j, k*128+p), so each SBUF partition receives
        # one contiguous NT*512B chunk per window DMA.
        feats_v = feats[:].rearrange("(j p k) f -> j p (k f)", p=P, k=NT)
        eq = mybir.AluOpType.is_equal
        mul = mybir.AluOpType.mult

        # Split one-hot builds between DVE (~285 ns) and GPSIMD (~361 ns) so
        # neither is the critical path.
        n_gp = (NT * 361) // (285 + 361)

        for j in range(WPC):
            ft = featp.tile([P, NT * D_FEAT], f32)
            nc.sync.dma_start(ft[:], feats_v[j])
            psS = psS_p.tile([P, P], f32)
            for k in range(NT):
                oh = ohp.tile([P, P], f32)
                c = (j * NT + k) * 2
                eng = nc.gpsimd if k < n_gp else nc.vector
                eng.tensor_scalar(
                    out=oh[:],
                    in0=iotat[:],
                    scalar1=metat[:, c : c + 1],
                    scalar2=metat[:, c + 1 : c + 2],
                    op0=eq,
                    op1=mul,
                )
                nc.tensor.matmul(
                    psS[:],
                    lhsT=ft[:, k * D_FEAT : (k + 1) * D_FEAT],
                    rhs=oh[:],
                    start=(k == 0),
                    stop=(k == NT - 1),
                )
            aggT = aggp.tile([P, P], f32)
            nc.scalar.copy(aggT[:], psS[:])
            psO = psO_p.tile([P, OUT_DIM], f32)
            nc.tensor.matmul(psO[:], lhsT=aggT[:], rhs=wbot[:], start=True, stop=False)
            nc.tensor.matmul(
                psO[:],
                lhsT=selft[:, j * P : (j + 1) * P],
                rhs=wtop[:],
                start=False,
                stop=True,
            )
            ob = obp.tile([P, OUT_DIM], f32)
            nc.scalar.copy(ob[:], psO[:])
            nc.sync.dma_start(outp[j * P : (j + 1) * P, :], ob[:])

    nc.compile()
    return nc


def _prep_inputs(self_feat, nbr_feat, relation_src_indices, W):
    """Host-side sharding: bucket edges by target window, pad each window to
    a common capacity T (multiple of 128), build per-core input arrays."""
    idx = np.asarray(relation_src_indices).astype(np.int64)
    feat = np.ascontiguousarray(np.asarray(nbr_feat, dtype=np.float32))
    E = idx.shape[0]

    win = idx >> 7  # global window id, 0..390
    counts_win = np.bincount(win, minlength=N_WIN)
    T = int(max(128, -(-int(counts_win.max()) // P) * P))
    NT = T // P

    order = np.argsort(win, kind="stable")
    sw = win[order]
    si = idx[order]
    starts = np.zeros(N_WIN, np.int64)
    starts[1:] = np.cumsum(counts_win)[:-1]
    rank = np.arange(E, dtype=np.int64) - starts[sw]
    # feats: partition-major placement within each window: edge rank r ->
    # row (r % 128) * NT + r // 128, so tile k / partition p holds edge rank
    # k*128+p (matching the meta layout) while each SBUF partition's window
    # data is one contiguous NT*512B chunk in DRAM.
    dest_feat = sw * T + (rank % P) * NT + rank // P
    # meta: tile-major (rank order) so the reshape below lands (j*NT+k, p).
    dest_meta = sw * T + rank

    feats_packed = np.zeros((N_WIN * T, D_FEAT), np.float32)
    feats_packed[dest_feat] = feat[order]

    lidx = np.full(N_WIN * T, -1.0, np.float32)
    lidx[dest_meta] = (si - (sw << 7)).astype(np.float32)

    cnt_node = np.bincount(idx, minlength=NODES_PAD).astype(np.float32)
    wv = np.zeros(N_WIN * T, np.float32)
    wv[dest_meta] = 1.0 / cnt_node[si]

    # meta[core, p, (j*NT+k)*2 + {0,1}] = lidx / weight of edge (j, k, p)
    lidx_t = lidx.reshape(N_CORES, WPC * NT, P).transpose(0, 2, 1)
    wv_t = wv.reshape(N_CORES, WPC * NT, P).transpose(0, 2, 1)
    meta = np.empty((N_CORES, P, WPC * NT * 2), np.float32)
    meta[:, :, 0::2] = lidx_t
    meta[:, :, 1::2] = wv_t

    selfp = np.zeros((NODES_PAD, D_FEAT), np.float32)
    selfp[:N_NODES] = np.asarray(self_feat, dtype=np.float32)
    selfT = np.ascontiguousarray(
        selfp.reshape(N_CORES, NPC, D_FEAT).transpose(0, 2, 1)
    )

    wrep = np.ascontiguousarray(np.asarray(W, dtype=np.float32))
    iota = np.ascontiguousarray(
        np.tile(np.arange(P, dtype=np.float32), (P, 1))
    )

    feats_c = feats_packed.reshape(N_CORES, WPC * T, D_FEAT)
    in_maps = [
        {
            "feats": np.ascontiguousarray(feats_c[c]),
            "meta": np.ascontiguousarray(meta[c]),
            "selfT": selfT[c],
            "wmat": wrep,
            "iota": iota,
        }
        for c in range(N_CORES)
    ]
    return T, in_maps


def kernel(self_feat, nbr_feat, relation_src_indices, W):
    from concourse.bass_utils import run_bass_kernel_spmd

    T, in_maps = _prep_inputs(self_feat, nbr_feat, relation_src_indices, W)

    nc = _prog_cache.get(T)
    if nc is None:
        nc = _build_program(T)
        _prog_cache[T] = nc

    res = run_bass_kernel_spmd(nc, in_maps, list(range(N_CORES)))
    out = np.concatenate([res.results[c]["outp"] for c in range(N_CORES)], axis=0)
    return np.ascontiguousarray(out[:N_NODES])
